# revision 1
# baseline (speedup 1.0000x reference)
"""Trainium2 Bass kernel for nn_Bridge_43739946942807.

Windowed GRU bridge: project [L,D,B] branch inputs to [L,D] rows, run a
T=64-step GRU over each row's trailing window (halo from in_buffer), then
BN -> LeakyReLU -> BN -> Dense back to [L,D].

Distribution: data-parallel over L across 8 cores (4096 rows each). The
T-1=63 halo rows each core needs are projected on the host (tiny) and
passed pre-transposed, so no device collectives are needed.

Per-core device program (all fp32):
  stage A: DMA input tiles [128, D, B]; 4 accumulating PE matmuls against
           scaled identities transpose+project them into xmT [D, Lb+63].
  stage B: per-gate flat projections XPf[g] = gru_W_g^T @ xmT  [8, Lb+63]
           (PE, N=512 blocks; gate bias folded into the PSUM->SBUF copy),
           then one SBUF->SBUF gather DMA per gate packs them into
           XP[g] [128, 319]: partition p = lane*16 + chunk, free = window
           offset, so each GRU step reads XP[g][:, t:t+256].
  stage C: 64 GRU steps on [128, 256] tiles (16 chunks x 256 chains):
           PSUM_z = I @ XPZ_t + Uz_bd @ h ; z = sigmoid(PSUM_z)   (PE+ACT)
           PSUM_r = I @ XPR_t + Ur_bd @ h ; r = sigmoid(PSUM_r)
           PSUM_h = Uh_bd @ h
           hh = tanh((PSUM_h + b1h)*r + XPH_t)                    (DVE+ACT)
           h  = hh + z*(h - hh)                                   (DVE)
           U*_bd are chunk-block-diagonal copies of the 8x8 U gate blocks.
  stage D: BN1 affine + LeakyReLU on h; output matmuls use per-chunk
           masked W_out tensors (BN2 folded in) so K stays full-width and
           partition bases stay quadrant-aligned; bias add; DMA out.
"""
import numpy as np

L = 32768
D = 128
B = 4
T = 64
H = 8
EPS = 1e-3
ALPHA = 0.05
NCORES = 8
LB = L // NCORES          # 4096 rows per core
CC = LB // 256            # 16 chunks per core


# ---------------------------------------------------------------- host prep

def _prep_consts(w_in, b_in, gru_W, gru_U, gru_b,
                 bn1_gamma, bn1_beta, bn1_mean, bn1_var,
                 bn2_gamma, bn2_beta, bn2_mean, bn2_var, w_out, b_out, C):
    P = 8 * C
    f = np.float32
    w = np.asarray(w_in, f)[:, 0]
    eyeD = np.eye(D, dtype=f)
    weye = np.stack([w[b] * eyeD for b in range(B)], axis=1)     # [D, B, D]

    U = np.asarray(gru_U, f)
    # partition layout p = i*C + c  (i = hidden lane, c = chunk)
    ubd = np.zeros((P, 3, P), f)
    for g in range(3):
        blk = U[:, g * H:(g + 1) * H]
        for c in range(C):
            for i2 in range(H):
                for i in range(H):
                    ubd[i2 * C + c, g, i * C + c] = blk[i2, i]

    eyeP = np.eye(P, dtype=f)

    b0 = np.asarray(gru_b, f)[0]
    b1 = np.asarray(gru_b, f)[1]
    a1 = np.asarray(bn1_gamma, f) / np.sqrt(np.asarray(bn1_var, f) + EPS)
    c1 = np.asarray(bn1_beta, f) - np.asarray(bn1_mean, f) * a1
    a2 = np.asarray(bn2_gamma, f) / np.sqrt(np.asarray(bn2_var, f) + EPS)
    c2 = np.asarray(bn2_beta, f) - np.asarray(bn2_mean, f) * a2

    pb = np.zeros((128, 8), f)
    pb[:, 6] = f(np.asarray(b_in, f).reshape(-1)[0])
    for i in range(H):
        sl = slice(i * C, i * C + C)
        pb[sl, 3] = b1[2 * H + i]
        pb[sl, 4] = a1[i]
        pb[sl, 5] = c1[i]
    gb8 = np.zeros((8, 3), f)
    gb8[:, 0] = b0[0:H] + b1[0:H]
    gb8[:, 1] = b0[H:2 * H] + b1[H:2 * H]
    gb8[:, 2] = b0[2 * H:3 * H]

    W_eff = a2[:, None] * np.asarray(w_out, f)
    b_eff = c2 @ np.asarray(w_out, f) + np.asarray(b_out, f)
    wout = np.zeros((P, C, D), f)
    for c in range(C):
        for i in range(H):
            wout[i * C + c, c, :] = W_eff[i]
    bout = np.tile(b_eff[None, :], (128, 1)).astype(f)

    gw = np.asarray(gru_W, f)
    return dict(weye=weye, gw=gw, ubd=ubd, eye=eyeP, pb=pb, gb8=gb8,
                wout=wout, bout=bout)


# ---------------------------------------------------------------- device IR

def _emit_kernel(ctx, tc, outs, ins, Lb):
    from concourse import mybir
    dt = mybir.dt.float32
    AF = mybir.ActivationFunctionType
    OP = mybir.AluOpType
    nc = tc.nc

    C = Lb // 256
    NT = Lb // 128
    P = 8 * C
    W = Lb + T - 1

    inp = ins["inp"]; haloT = ins["haloT"]; weye = ins["weye"]
    gw = ins["gw"]; ubd = ins["ubd"]; eye = ins["eye"]; pb = ins["pb"]
    wout = ins["wout"]; bout = ins["bout"]; gb8 = ins["gb8"]
    outd = outs["out"]

    cpool = ctx.enter_context(tc.tile_pool(name="consts", bufs=1))
    weye_t = cpool.tile([D, B, D], dt)
    nc.sync.dma_start(out=weye_t[:], in_=weye[:])
    gw_t = cpool.tile([D, 3 * H], dt)
    nc.sync.dma_start(out=gw_t[:], in_=gw[:])
    ubd_t = cpool.tile([P, 3, P], dt)
    nc.sync.dma_start(out=ubd_t[:], in_=ubd[:])
    eye_t = cpool.tile([P, P], dt)
    nc.sync.dma_start(out=eye_t[:], in_=eye[:])
    pb_t = cpool.tile([128, 8], dt)
    nc.sync.dma_start(out=pb_t[:], in_=pb[:])
    gb8_t = cpool.tile([8, 3], dt)
    nc.sync.dma_start(out=gb8_t[:], in_=gb8[:])
    wout_t = cpool.tile([P, C, D], dt)
    nc.sync.dma_start(out=wout_t[:], in_=wout[:])
    bout_t = cpool.tile([128, D], dt)
    nc.sync.dma_start(out=bout_t[:], in_=bout[:])

    xm_pool = ctx.enter_context(tc.tile_pool(name="xmT", bufs=1))
    xmT = xm_pool.tile([D, W], dt)
    nc.sync.dma_start(out=xmT[:, 0:T - 1], in_=haloT[:])

    # ---- stage A: project + transpose input tiles into xmT
    with tc.tile_pool(name="inp", bufs=6) as ipool, \
         tc.tile_pool(name="tp_ps", bufs=2, space="PSUM") as tppool:
        for g in range(NT):
            it = ipool.tile([128, D, B], dt)
            nc.sync.dma_start(out=it[:], in_=inp[g * 128:(g + 1) * 128, :, :])
            pt = tppool.tile([128, 128], dt)
            for b in range(B):
                nc.tensor.matmul(pt[:], lhsT=it[:, :, b], rhs=weye_t[:, b, :],
                                 start=(b == 0), stop=(b == B - 1))
            nc.scalar.activation(xmT[:, T - 1 + g * 128: T - 1 + (g + 1) * 128],
                                 pt[:], AF.Identity, bias=pb_t[0:128, 6:7])

    # ---- stage B: flat per-gate projections, then gather into packed layout
    xp_pool = ctx.enter_context(tc.tile_pool(name="xp", bufs=1))
    XP = []
    XPf = []
    for g in range(3):
        xpt = xp_pool.tile([P, 320], dt, tag=f"xp{g}", name=f"xp{g}")
        XP.append(xpt)
        xpf = xp_pool.tile([8, W], dt, tag=f"xpf{g}", name=f"xpf{g}")
        XPf.append(xpf)
    NB = (W + 511) // 512
    with tc.tile_pool(name="xp_ps", bufs=6, space="PSUM") as xppool:
        for g in range(3):
            for b in range(NB):
                o = b * 512
                n = min(512, W - o)
                xps = xppool.tile([8, 512], dt, tag="xps")
                nc.tensor.matmul(xps[:, 0:n],
                                 lhsT=gw_t[:, g * H:(g + 1) * H],
                                 rhs=xmT[:, o:o + n], start=True, stop=True)
                dst = XPf[g][:, o:o + n]
                if g == 2:
                    nc.scalar.activation(dst, xps[:, 0:n], AF.Identity,
                                         bias=gb8_t[0:8, g:g + 1])
                else:
                    nc.vector.tensor_scalar(dst, xps[:, 0:n],
                                            gb8_t[0:8, g:g + 1],
                                            None, op0=OP.add)
            # gather into chunk-packed layout: dest partition p = i*C + c
            src = XPf[g][:].copy()
            v = src.ap
            s0 = v.to_list()[0][0]
            v.clear()
            v.extend([[s0, 8], [256, C], [1, 319]])
            src.ap = v
            nc.sync.dma_start(out=XP[g][:, 0:319], in_=src)
    XPZ, XPR, XPH = XP

    # ---- stage C: recurrence
    spool = ctx.enter_context(tc.tile_pool(name="state", bufs=1))
    h = spool.tile([P, 256], dt)
    nc.vector.memset(h[:], 0.0)

    with tc.tile_pool(name="gates", bufs=3) as gpool, \
         tc.tile_pool(name="rec_ps", bufs=2, space="PSUM") as rpool:
        for t in range(T):
            pz = rpool.tile([P, 256], dt, tag="pz")
            nc.tensor.matmul(pz[:], lhsT=eye_t[:], rhs=XPZ[:, t:t + 256],
                             start=True, stop=False)
            nc.tensor.matmul(pz[:], lhsT=ubd_t[:, 0, :], rhs=h[:],
                             start=False, stop=True)
            pr = rpool.tile([P, 256], dt, tag="pr")
            nc.tensor.matmul(pr[:], lhsT=eye_t[:], rhs=XPR[:, t:t + 256],
                             start=True, stop=False)
            nc.tensor.matmul(pr[:], lhsT=ubd_t[:, 1, :], rhs=h[:],
                             start=False, stop=True)
            ph = rpool.tile([P, 256], dt, tag="ph")
            nc.tensor.matmul(ph[:], lhsT=ubd_t[:, 2, :], rhs=h[:],
                             start=True, stop=True)

            z = gpool.tile([P, 256], dt, tag="z")
            nc.scalar.activation(z[:], pz[:], AF.Sigmoid)
            r = gpool.tile([P, 256], dt, tag="r")
            nc.scalar.activation(r[:], pr[:], AF.Sigmoid)

            t1 = gpool.tile([P, 256], dt, tag="t1")
            nc.vector.scalar_tensor_tensor(t1[:], in0=ph[:],
                                           scalar=pb_t[0:P, 3:4], in1=r[:],
                                           op0=OP.add, op1=OP.mult)
            t2 = gpool.tile([P, 256], dt, tag="t2")
            nc.vector.tensor_add(t2[:], t1[:], XPH[:, t:t + 256])
            hh = gpool.tile([P, 256], dt, tag="hh")
            nc.scalar.activation(hh[:], t2[:], AF.Tanh)

            d = gpool.tile([P, 256], dt, tag="d")
            nc.vector.tensor_sub(d[:], h[:], hh[:])
            e = gpool.tile([P, 256], dt, tag="e")
            nc.vector.tensor_mul(e[:], d[:], z[:])
            nc.vector.tensor_add(h[:], hh[:], e[:])

    # ---- stage D: BN1 -> LeakyReLU -> (BN2+dense folded) -> out
    with tc.tile_pool(name="post", bufs=2) as ppool, \
         tc.tile_pool(name="out_ps", bufs=4, space="PSUM") as opool, \
         tc.tile_pool(name="out_sb", bufs=4) as ospool:
        gbn = ppool.tile([P, 256], dt)
        nc.vector.tensor_scalar(gbn[:], h[:], pb_t[0:P, 4:5], pb_t[0:P, 5:6],
                                op0=OP.mult, op1=OP.add)
        lra = ppool.tile([P, 256], dt)
        nc.scalar.activation(lra[:], gbn[:], AF.Copy, scale=ALPHA)
        lr = ppool.tile([P, 256], dt)
        nc.vector.tensor_max(lr[:], gbn[:], lra[:])
        for n2 in range(2):
            for c in range(C):
                po = opool.tile([128, D], dt)
                nc.tensor.matmul(po[:],
                                 lhsT=lr[:, n2 * 128:(n2 + 1) * 128],
                                 rhs=wout_t[:, c, :], start=True, stop=True)
                ot = ospool.tile([128, D], dt)
                nc.vector.tensor_add(ot[:], po[:], bout_t[:])
                nc.sync.dma_start(
                    out=outd[c * 256 + n2 * 128: c * 256 + (n2 + 1) * 128, :],
                    in_=ot[:])


# ---------------------------------------------------------------- entry

_CACHE = {}


def _build_nc():
    import concourse.bacc as bacc
    from concourse import mybir
    from concourse.tile import TileContext
    from contextlib import ExitStack

    dt = mybir.dt.float32
    nc = bacc.Bacc("TRN2", target_bir_lowering=False, debug=False,
                   num_devices=NCORES)
    C = CC
    P = 8 * C
    shapes = dict(inp=[LB, D, B], haloT=[D, T - 1], weye=[D, B, D],
                  gw=[D, 3 * H], ubd=[P, 3, P], eye=[P, P], pb=[128, 8],
                  gb8=[8, 3], wout=[P, C, D], bout=[128, D])
    ins = {k: nc.dram_tensor(k, v, dt, kind="ExternalInput").ap()
           for k, v in shapes.items()}
    outs = {"out": nc.dram_tensor("out", [LB, D], dt,
                                  kind="ExternalOutput").ap()}
    with TileContext(nc) as tc, ExitStack() as ctx:
        _emit_kernel(ctx, tc, outs, ins, LB)
    nc.finalize()
    return nc


def kernel(inputs, in_buffer, w_in, b_in, gru_W, gru_U, gru_b,
           bn1_gamma, bn1_beta, bn1_mean, bn1_var,
           bn2_gamma, bn2_beta, bn2_mean, bn2_var, w_out, b_out):
    from concourse.bass_utils import run_bass_kernel_spmd

    f = np.float32
    inputs = np.ascontiguousarray(np.asarray(inputs, f))
    in_buffer = np.asarray(in_buffer, f)
    consts = _prep_consts(w_in, b_in, gru_W, gru_U, gru_b,
                          bn1_gamma, bn1_beta, bn1_mean, bn1_var,
                          bn2_gamma, bn2_beta, bn2_mean, bn2_var,
                          w_out, b_out, CC)

    w = np.asarray(w_in, f)[:, 0]
    b0 = f(np.asarray(b_in, f).reshape(-1)[0])
    in_maps = []
    for ci in range(NCORES):
        s = ci * LB
        if ci == 0:
            halo = in_buffer
        else:
            halo = inputs[s - (T - 1):s] @ w + b0
        m = dict(inp=np.ascontiguousarray(inputs[s:s + LB]),
                 haloT=np.ascontiguousarray(halo.T.astype(f)))
        m.update(consts)
        in_maps.append(m)

    if "nc" not in _CACHE:
        _CACHE["nc"] = _build_nc()
    nc = _CACHE["nc"]

    res = run_bass_kernel_spmd(nc, in_maps, list(range(NCORES)))
    out = np.concatenate([res.results[ci]["out"] for ci in range(NCORES)],
                         axis=0)

    # out_buffer: last T-1 projected rows (host; trivial)
    out_buffer = (inputs[L - (T - 1):] @ w + b0).astype(f)
    return out, out_buffer
